# revision 1
# baseline (speedup 1.0000x reference)
"""Trainium2 Bass kernel for the ragged-sequence attention problem.

Math (per batch row):
    u      = tanh(h @ W.T + b)          h: [L, H]
    s      = u @ u_s                    masked to length, then softmax
    v      = sum_l alpha_l * h_l

Strategy: pure data parallel over the batch axis, 8 NeuronCores x 32 rows.
Per core, per batch row (l-partition mapping l = 16p + t):
  - DMA h as [128, 16, 240] f32 (3.8KB contiguous runs), cast to bf16 (DVE)
    into 242-wide tiles holding a ones column (accumulates the softmax
    denominator inside the v matmul; 242 keeps 4B alignment for FWL).
  - PE-transpose h tiles into ht0/ht1 bf16 (contraction over H needs H on
    partitions); all transposes read 128 columns so fast-weight-load kicks in.
  - u-matmul: stationary W.T chunks (zero-padded to two full 128-col
    m-chunks), moving ht, N=512 groups -> psum ut [o, l].
  - tanh+bias on ScalarE straight out of PSUM into bf16.
  - scores: stationary u_s column (1-col weight load), moving ut tiles;
    col-tiled (tile_position) so the 4 L-groups land on psum partitions
    0/32/64/96 and evacuate in ONE DVE copy.
  - reshape scores to [128, 16] via a DRAM bounce (DMA cannot read PSUM and
    engines cannot cross partitions), exp (ScalarE), multiply by a
    host-precomputed length mask (DVE).
  - v-matmul: stationary weight column w[:, t], moving h bf16 tiles
    (the ones column makes out[240] the denominator).
The batch loop is software-pipelined 3 deep (exp/mask/v-matmul of batch b
issue after the PE-dense front of batch b+3) so the in-order engines never
stall on the softmax chain.  Host divides by the denominator and
concatenates the 8 shards.
"""

import sys

import numpy as np

sys.path.insert(0, "/opt/trn_rl_repo")

import concourse.bass as bass  # noqa: E402
import concourse.mybir as mybir  # noqa: E402
import concourse.tile as tile  # noqa: E402
from concourse.masks import make_identity  # noqa: E402
from concourse.bass_utils import run_bass_kernel_spmd  # noqa: E402
import bass_rust as _br  # noqa: E402

N_CORES = 8
B, L, H = 256, 2048, 240
BPC = B // N_CORES        # batch rows per core
NT = L // 128             # 16 l-tiles of 128
NG = 4                    # l-groups of 512
GSZ = 512
H0, H1 = 128, 112         # H split across partitions
HB = H + 1                # ones column position (out width)
HTW = 242                 # h tile stride in hbf: 240 ch + ones + pad (4B align)
HBW = NT * HTW + 16       # hbf width, padded so 128-col reads stay in bounds
WP = 256                  # W stationary padded to two full 128-col m-chunks
F32 = mybir.dt.float32
BF16 = mybir.dt.bfloat16
AF = mybir.ActivationFunctionType


_MAXW = 1  # sync waits kept on an instruction; the rest move to nops


class _TC(tile.TileContext):
    """Walrus in this container caps sync-wait commands per instruction
    ("Too many sync wait commands"), but Tile freely attaches one wait per
    producer semaphore.  After scheduling, hoist excess waits onto dedicated
    single-wait nops inserted just before the instruction on its engine."""

    def schedule_and_allocate(self, validate_deps=False):
        ret = super().schedule_and_allocate(validate_deps)
        self._split_excess_waits()
        return ret

    def _split_excess_waits(self):
        nc = self.nc
        n_split = 0
        for fn in nc.m.functions:
            for bb in fn.blocks:
                insts = bb.instructions
                i = 0
                while i < len(insts):
                    inst = insts[i]
                    si = getattr(inst, "sync_info", None)
                    waits = list(si.on_wait) if si is not None else []
                    if len(waits) > _MAXW:
                        si.on_wait = waits[-_MAXW:]
                        inst.sync_info = si
                        for w in waits[:-_MAXW]:
                            nop = mybir.InstNoOp(
                                name=f"waitsplit-{n_split}", ins=[], outs=[])
                            n_split += 1
                            nop.engine = inst.engine
                            nop.sync_info = _br.SyncInfo(
                                on_wait=[w], on_update=[])
                            nc.register_instruction(nop, overwrite=True)
                            insts.insert(i, nop)
                            i += 1
                    i += 1


OW = HB  # out row: 0:240 v_acc, 240 softmax denominator


def build():
    nc = bass.Bass("TRN2", target_bir_lowering=False, debug=False,
                   num_devices=N_CORES)
    h_d = nc.declare_dram_parameter("h", [BPC, L, H], F32, isOutput=False)
    wt_d = nc.declare_dram_parameter("wt", [H, WP], F32, isOutput=False)
    us_d = nc.declare_dram_parameter("usT", [H, BPC], F32, isOutput=False)
    b_d = nc.declare_dram_parameter("bias", [H, 1], F32, isOutput=False)
    m_d = nc.declare_dram_parameter("mask", [BPC, 128, NT], F32, isOutput=False)
    o_d = nc.declare_dram_parameter("out", [BPC, 1, OW], F32, isOutput=True)

    with _TC(nc) as tc:
        with (
            tc.tile_pool(name="consts", bufs=1) as cp,
            tc.tile_pool(name="hf", bufs=3) as hfp,
            tc.tile_pool(name="hb", bufs=5) as hbp,
            tc.tile_pool(name="ht", bufs=2) as htp,
            tc.tile_pool(name="ut", bufs=3) as utp,
            tc.tile_pool(name="small", bufs=5) as sp,
            tc.tile_pool(name="pt", bufs=1, space="PSUM") as ptp,
            tc.tile_pool(name="pu", bufs=2, space="PSUM") as pup,
            tc.tile_pool(name="psv", bufs=1, space="PSUM") as psvp,
            tc.tile_pool(name="dscr", bufs=2, space="DRAM") as dp,
        ):
            ident = cp.tile([128, 128], BF16)
            make_identity(nc, ident[:])

            wtf0 = cp.tile([H0, WP], F32)
            wtf1 = cp.tile([H1, WP], F32)
            nc.sync.dma_start(wtf0[:], wt_d.ap()[0:H0, :])
            nc.sync.dma_start(wtf1[:], wt_d.ap()[H0:H, :])
            wtb0 = cp.tile([H0, WP], BF16)
            wtb1 = cp.tile([H1, WP], BF16)
            nc.vector.tensor_copy(wtb0[:], wtf0[:])
            nc.vector.tensor_copy(wtb1[:], wtf1[:])

            usf0 = cp.tile([H0, BPC], F32)
            usf1 = cp.tile([128, BPC], F32)
            nc.sync.dma_start(usf0[:], us_d.ap()[0:H0, :])
            nc.gpsimd.memset(usf1[96:128, :], 0.0)
            nc.sync.dma_start(usf1[0:H1, :], us_d.ap()[H0:H, :])
            usb0 = cp.tile([H0, BPC], BF16)
            usb1 = cp.tile([128, BPC], BF16)
            nc.vector.tensor_copy(usb0[:], usf0[:])
            nc.vector.tensor_copy(usb1[:], usf1[:])

            b0 = cp.tile([H0, 1], F32)
            b1 = cp.tile([128, 1], F32)
            nc.sync.dma_start(b0[:], b_d.ap()[0:H0, :])
            nc.gpsimd.memset(b1[96:128, :], 0.0)
            nc.sync.dma_start(b1[0:H1, :], b_d.ap()[H0:H, :])

            def stage_front(b):
                """DMA-in, cast, transpose, u-matmul, tanh, scores.
                Returns state consumed by stage_tail."""
                hf = hfp.tile([128, NT * H], F32, tag="hf")
                # l = 16*p + t: contiguous 3.8KB runs per partition per DMA
                hview = h_d.ap()[b].rearrange("(p t) c -> p t c", t=NT)
                hfv = hf[:].rearrange("p (t c) -> p t c", c=H)
                for q in range(4):
                    nc.sync.dma_start(hfv[:, q * 4:(q + 1) * 4, :],
                                      hview[:, q * 4:(q + 1) * 4, :])

                hbf = hbp.tile([128, HBW], BF16, tag="hbf")
                hbv = hbf[:, 0:NT * HTW].rearrange("p (t c) -> p t c", c=HTW)
                nc.gpsimd.memset(hbv[:, :, H:HB], 1.0)
                for cg in range(NG):
                    cs = slice(cg * 4, (cg + 1) * 4)
                    nc.vector.tensor_copy(hbv[:, cs, 0:H], hfv[:, cs, :])

                msk = sp.tile([128, NT], F32, tag="msk")
                nc.gpsimd.dma_start(msk[:], m_d.ap()[b])

                ht0 = htp.tile([H0, L], BF16, tag="ht0")
                ht1 = htp.tile([H1, L], BF16, tag="ht1")
                ps = psvp.tile([128, GSZ], F32, tag="ps")

                for g in range(NG):
                    pt0 = ptp.tile([H0, GSZ], BF16, tag="pt0")
                    pt1 = ptp.tile([128, GSZ], BF16, tag="pt1")
                    for t4 in range(4):
                        t = g * 4 + t4
                        base = t * HTW
                        nc.tensor.transpose(
                            pt0[:, t4 * 128:(t4 + 1) * 128],
                            hbf[:, base:base + 128], ident[:])
                        nc.tensor.transpose(
                            pt1[:, t4 * 128:(t4 + 1) * 128],
                            hbf[:, base + H0:base + H0 + 128], ident[:])
                    gs = slice(g * GSZ, (g + 1) * GSZ)
                    nc.vector.tensor_copy(ht0[:, gs], pt0[:])
                    nc.vector.tensor_copy(ht1[:, gs], pt1[0:H1, :])

                    pu0 = pup.tile([H0, GSZ], F32, tag="pu0")
                    pu1 = pup.tile([128, GSZ], F32, tag="pu1")
                    nc.tensor.matmul(pu0[:], wtb0[:, 0:H0], ht0[:, gs],
                                     start=True, stop=False)
                    nc.tensor.matmul(pu0[:], wtb1[:, 0:H0], ht1[:, gs],
                                     start=False, stop=True)
                    nc.tensor.matmul(pu1[:], wtb0[:, H0:WP], ht0[:, gs],
                                     start=True, stop=False)
                    nc.tensor.matmul(pu1[:], wtb1[:, H0:WP], ht1[:, gs],
                                     start=False, stop=True)

                    ut0 = utp.tile([H0, GSZ], BF16, tag="ut0")
                    ut1 = utp.tile([128, GSZ], BF16, tag="ut1")
                    nc.scalar.activation(ut0[:], pu0[:], AF.Tanh, bias=b0[:])
                    nc.scalar.activation(ut1[:], pu1[:], AF.Tanh, bias=b1[:])

                    nc.tensor.matmul(ps[32 * g:32 * g + 1, :],
                                     usb0[:, b:b + 1], ut0[:],
                                     start=True, stop=False,
                                     tile_position=(0, 32 * g))
                    nc.tensor.matmul(ps[32 * g:32 * g + 1, :],
                                     usb1[:, b:b + 1], ut1[:],
                                     start=False, stop=True,
                                     tile_position=(0, 32 * g))

                # scores evac + reshape + exp + mask: all off the PE, so
                # they overlap the next batch's front on DVE/ACT/DMA
                s4 = sp.tile([128, GSZ], F32, tag="s4")
                nc.vector.tensor_copy(s4[0:97, :], ps[0:97, :])

                dscr = dp.tile([NG, GSZ], F32, tag="dscr")
                for g in range(NG):
                    nc.sync.dma_start(dscr[g:g + 1, :],
                                      s4[32 * g:32 * g + 1, :])
                scr = sp.tile([128, NT], F32, tag="scr")
                nc.sync.dma_start(
                    scr[:].rearrange("p (g c) -> p g c", g=NG),
                    dscr[:].rearrange("g (c p) -> p g c", p=128))

                return b, hbf, scr, msk

            def stage_tail(state):
                """exp/mask + weighted sum (v) for a batch whose scores
                finished their reshape several fronts ago."""
                b, hbf, scr, msk = state
                e = sp.tile([128, NT], F32, tag="e")
                nc.scalar.activation(e[:], scr[:], AF.Exp)
                w = sp.tile([128, NT], BF16, tag="w")
                nc.vector.tensor_mul(w[:], e[:], msk[:])
                pv = psvp.tile([1, OW], F32, tag="pv")
                for t in range(NT):
                    nc.tensor.matmul(pv[:], w[:, t:t + 1],
                                     hbf[:, t * HTW:t * HTW + OW],
                                     start=(t == 0), stop=(t == NT - 1))
                orow = sp.tile([1, OW], F32, tag="orow")
                nc.vector.tensor_copy(orow[:], pv[:])
                nc.gpsimd.dma_start(o_d.ap()[b], orow[:])

            # software pipeline: batch b's PE-light tail is issued after
            # batch b+1's PE-dense front, so the in-order PE never stalls
            # waiting for batch b's softmax
            pending = []
            for b in range(BPC):
                if len(pending) >= 3:
                    stage_tail(pending.pop(0))
                pending.append(stage_front(b))
            for st in pending:
                stage_tail(st)

    return nc


_NC_CACHE = None


def _get_nc():
    global _NC_CACHE
    if _NC_CACHE is None:
        _NC_CACHE = build()
    return _NC_CACHE


def _prep_in_maps(short_perference, current_perference, W, bvec, length_input):
    h = np.asarray(short_perference, dtype=np.float32)[0]      # [B, L, H]
    us = np.asarray(current_perference, dtype=np.float32)[0]   # [B, H]
    W = np.asarray(W, dtype=np.float32)
    bvec = np.asarray(bvec, dtype=np.float32)
    lens = np.asarray(length_input).astype(np.int64)

    wt = np.zeros((H, WP), dtype=np.float32)
    wt[:, :H] = W.T                                            # [H(k), o padded]
    bias = np.ascontiguousarray(bvec.reshape(H, 1))

    p = np.arange(128)[:, None]                                # [128, 1]
    t = np.arange(NT)[None, :]                                 # [1, NT]
    pos = (NT * p + t)                                         # l = 16p + t

    in_maps = []
    for c in range(N_CORES):
        sl = slice(c * BPC, (c + 1) * BPC)
        mask = (pos[None, :, :] < lens[sl, None, None]).astype(np.float32)
        in_maps.append({
            "h": np.ascontiguousarray(h[sl]),
            "wt": wt,
            "usT": np.ascontiguousarray(us[sl].T),
            "bias": bias,
            "mask": np.ascontiguousarray(mask),
        })
    return in_maps


def run(short_perference, current_perference, W, b, length_input,
        trace=False, **run_kwargs):
    nc = _get_nc()
    in_maps = _prep_in_maps(short_perference, current_perference, W, b,
                            length_input)
    res = run_bass_kernel_spmd(nc, in_maps, list(range(N_CORES)),
                               trace=trace, **run_kwargs)
    outs = []
    for c in range(N_CORES):
        o = np.asarray(res.results[c]["out"], dtype=np.float32)
        o = o[:, ::32, :].sum(axis=1)                            # [BPC, 241]
        outs.append(o[:, :H] / o[:, H:H + 1])
    v = np.concatenate(outs, axis=0)                             # [B, H]
    return v, res


def kernel(short_perference, current_perference, W, b, current_batch,
           length_input):
    v, _ = run(short_perference, current_perference, W, b, length_input)
    return v.astype(np.float32)

